# revision 47
# baseline (speedup 1.0000x reference)
"""DivergentAttention Trainium2 kernel (8 NeuronCores, Bass/Tile).

Problem: GPT-2 style causal self-attention (B=2, S=2048, D=1024, H=16,
hd=64) where heads 0/1/2 re-weight their attention toward a token region
(first/middle/last third of the sequence) with factor 1.6 and renormalize.

Key identity: softmax(s)*m / sum(softmax(s)*m) == softmax(s + log m), so the
per-head region reweight folds into an additive per-(head, key-position)
bias on the scores -- no second normalization pass needed.  Scores are small
(|s|<~5) so the max-subtraction pass is skipped entirely.

Sharding: tensor-parallel over (batch, head-group): core c handles batch
c//4 and heads [4*(c%4), 4*(c%4)+4).  Each core computes the QKV projection
for its 4 heads, full causal attention, and its partial c_proj; the host
sums the 8 partials (fp32) and adds c_proj_b.

All matmul inputs are bf16 (error budget allows it; bf16 runs the PE at one
row per output column for every width, unlike fp32r which needs N>=256).

Attention structure per head (the big change vs the v1 kernel):
  - scoresT [sk-tile 128, q] = kT.T @ qT as before, exp'd (ScalarE, scale
    1/8, bias log mult) into a per-head PERSISTENT attn buffer at_sb
    [128, 17408] bf16 holding the whole causal triangle.
  - AV is FLIPPED: out[q-tile 128, 65] = attnT(stationary).T @ v_aug --
    65 output columns per (q-tile, sk-tile) pair instead of 512-wide
    rows, halving AV PE time.  v_aug column 64 is ones, so column 64 of
    the output is the softmax denominator per q ROW -- normalization
    becomes a per-partition scalar multiply (DVE reciprocal + mult), no
    partition-broadcast DMA bounce.
  - The normalized per-head output O' [q, 64] bf16 is transposed back to
    [64*(h%2)+d, q] for c_proj with a PE transpose (identity moving
    operand), 128 rows per q-tile.
  - c_proj: two K=128 matmuls per output tile against pw2 [128, 2, D].

Window schedule (PE-balance: exp on ScalarE is the per-head pacing limit,
so PE filler work is spread into the exp-bound windows):
  w0: head 0 + one v-projection tile per round (pr-tag PSUM).
  w1: head 1 + the deferred qk projection round for heads 2/3 (8 groups,
      rounds 0..7) + pair-0 transposes.
  w2: head 3 + head-2 score tiles 0..4 pulled ahead (rounds 11..15).
  w3: head 2 (scores 5..15) + pair-1 transposes + c_proj, software-
      pipelined (transpose at round t-2, c_proj at round t-3) so the PE
      never waits on the DVE/ScalarE drains.
"""

import numpy as np
import ml_dtypes

import concourse.bass as bass
import concourse.tile as tile
from concourse import mybir
from concourse import bass_utils, bass2jax

# ---------------------------------------------------------------- constants
B, S, D, H, HD = 2, 2048, 1024, 16, 64
NCORES = 8
HPC = 4              # heads per core
GROUPS = 4           # head groups
FOCUS = 1.6
HEAD_REGION = {0: 0, 1: 1, 2: 2}
NT = S // 128        # 16 sk/q tiles
KO = D // 128        # 8 contraction chunks
BF = mybir.dt.bfloat16
F32 = mybir.dt.float32

# column offset of tile t's rows inside the per-head attn buffer
OFF = [0] * (NT + 1)
for _t in range(NT):
    OFF[_t + 1] = OFF[_t] + (S - 128 * _t)
ATW = OFF[NT]        # 17408

# ------------------------------------------------- walrus multi-wait fixup
# This container's walrus accepts only ONE sync-wait per TPB instruction,
# but Tile attaches one wait per dependency proc.  Rewrite the BIR JSON just
# before walrus: hoist all-but-one wait of a multi-wait instruction onto
# standalone same-engine NoOps inserted immediately before it (same-engine
# program order is preserved, so semantics are unchanged).
try:
    import orjson as _json
except ImportError:  # pragma: no cover
    import json as _json

_orig_compile_bir_kernel = bass_utils.compile_bir_kernel
_wfix_counter = [0]


def _fix_bir(bir_json):
    d = _json.loads(bir_json)
    changed = False
    for fn in d.get("functions", []):
        for blk in fn.get("blocks", []):
            out = []
            for inst in blk.get("instructions", []):
                si = inst.get("sync_info")
                if si:
                    waits = si.get("on_wait") or []
                    if len(waits) > 1:
                        changed = True
                        for w in waits[:-1]:
                            _wfix_counter[0] += 1
                            nop = {
                                "engine": inst["engine"],
                                "ins": [],
                                "name": f"I-wfix-{_wfix_counter[0]}",
                                "opcode": "NoOp",
                                "outs": [],
                                "sync_info": {"on_update": [], "on_wait": [w]},
                            }
                            if "debug" in inst:
                                nop["debug"] = inst["debug"]
                            out.append(nop)
                        si["on_wait"] = waits[-1:]
                out.append(inst)
            blk["instructions"] = out
    return _json.dumps(d) if changed else bir_json


def _patched_compile_bir_kernel(bir_json, tmpdir, neff_name="file.neff"):
    return _orig_compile_bir_kernel(_fix_bir(bir_json), tmpdir, neff_name=neff_name)


def _install_waitfix():
    bass_utils.compile_bir_kernel = _patched_compile_bir_kernel
    bass2jax.compile_bir_kernel = _patched_compile_bir_kernel


_install_waitfix()

# ---------------------------------------------------------------- program


def build_program():
    """One SPMD Bass program; per-core differences come in via inputs."""
    nc = bass.Bass()

    hiddenT = nc.dram_tensor("hiddenT", [D, S], BF, kind="ExternalInput")
    w_qkv = nc.dram_tensor("w_qkv", [D, 768], BF, kind="ExternalInput")
    bqk = nc.dram_tensor("bqk", [128, 4], F32, kind="ExternalInput")
    bv_rep = nc.dram_tensor("bv_rep", [128, 256], F32, kind="ExternalInput")
    projw = nc.dram_tensor("projw", [128, 2, D], BF, kind="ExternalInput")
    diag_mask = nc.dram_tensor("diag_mask", [128, 128], BF, kind="ExternalInput")
    ident = nc.dram_tensor("ident", [128, 128], BF, kind="ExternalInput")
    logmult = nc.dram_tensor("logmult", [128, HPC, NT], F32, kind="ExternalInput")
    out = nc.dram_tensor("out", [S, D], BF, kind="ExternalOutput")

    with tile.TileContext(nc) as tc:
        with tc.tile_pool(name="persist", bufs=1) as persist, \
             tc.tile_pool(name="atp", bufs=2) as atp, \
             tc.tile_pool(name="stgp", bufs=2) as stgp, \
             tc.tile_pool(name="recp", bufs=4) as recp, \
             tc.tile_pool(name="outp", bufs=4) as outp:

            # ---- persistent SBUF ----
            # hT and w_sb die after window 1 (last v-round / qk-r1 reads);
            # head 2's attention buffer manually reuses their bytes via
            # overlapping views of one scratch tile (Tile's AP-range WAR
            # tracking orders the reuse).
            scratch = persist.tile([128, 23552], BF)      # 5.75 MB
            hT = scratch[:, 0:KO * S].rearrange("p (ko s) -> p ko s", s=S)
            w_sb = scratch[:, KO * S:KO * S + KO * 768].rearrange(
                "p (ko n) -> p ko n", n=768)
            at2_view = scratch[:, 0:ATW]
            at3_sb = persist.tile([128, ATW], BF)         # head 3, no WAR
            qk_sb = persist.tile([128, 4, S], BF)         # 2 MB
            v_sb = persist.tile([128, NT, HPC, 65], BF)   # ~1 MB
            ao2 = persist.tile([128, 2, S], BF)           # attn_outT, 1 MB
            bqk_sb = persist.tile([128, 4], F32)
            bv_sb = persist.tile([128, 256], F32)
            pw_sb = persist.tile([128, 2, D], BF)         # 0.5 MB
            dm_sb = persist.tile([128, 128], BF)
            id_sb = persist.tile([128, 128], BF)
            lm_sb = persist.tile([128, HPC, NT], F32)

            nc.sync.dma_start(bqk_sb, bqk[:, :])
            nc.vector.memset(v_sb[:, :, :, 64:65], 1.0)

            # input loads: hT alternates the SP and ACT HWDGE queues (ACT
            # is idle until the first exp), w + small tensors ride the
            # GPSIMD SWDGE queue.
            hT_src = hiddenT.rearrange("(ko p) s -> p ko s", p=128)
            w_src = w_qkv.rearrange("(ko p) n -> p ko n", p=128)
            # first w slice + first hT chunk in small pieces so the first
            # matmul can fire as early as possible
            nc.sync.dma_start(w_sb[:, 0, 0:128], w_src[:, 0, 0:128])
            for pc in range(4):
                nc.sync.dma_start(hT[:, 0, 512 * pc:512 * pc + 512],
                                  hT_src[:, 0, 512 * pc:512 * pc + 512])
            nc.gpsimd.dma_start(w_sb[:, 0, 128:768], w_src[:, 0, 128:768])
            for ko in range(1, KO):
                q = nc.sync if ko % 2 == 0 else nc.scalar
                q.dma_start(hT[:, ko, :], hT_src[:, ko, :])
                nc.gpsimd.dma_start(w_sb[:, ko, :], w_src[:, ko, :])
                if ko == 1:
                    nc.sync.dma_start(bv_sb, bv_rep[:, :])
                    nc.gpsimd.dma_start(dm_sb, diag_mask[:, :])
                    nc.gpsimd.dma_start(id_sb, ident[:, :])
                    nc.gpsimd.dma_start(lm_sb, logmult[:, :, :])
                    nc.gpsimd.dma_start(pw_sb, projw[:, :, :])

            # ========== qk projection round 0 (heads 0/1: nt 0 and 2) =====
            # ko (contraction) outer with 4 resident PSUM groups per pass;
            # two passes so the first pass's banks drain (DVE) while the
            # second computes, letting the attention pools start without
            # waiting for the full round to drain.
            with tc.tile_pool(name="p1ps", bufs=8, space="PSUM") as p1ps:
                # prologue computes ONLY q of heads 0/1 (nt 0): scores for
                # k-tile t need just one 512-column k chunk, so the k
                # projection is deferred into window 0 as PE filler and
                # the first exp fires ~7us earlier.
                ps4 = [p1ps.tile([128, 512], F32, tag="g", name=f"q0{i}")
                       for i in range(4)]
                for ko in range(KO):
                    for i in range(4):
                        nc.tensor.matmul(
                            ps4[i],
                            w_sb[:, ko, 0:128],
                            hT[:, ko, 512 * i:512 * i + 512],
                            start=(ko == 0), stop=(ko == KO - 1),
                        )
                for i in range(4):
                    nc.vector.tensor_scalar_add(
                        qk_sb[:, 0, 512 * i:512 * i + 512], ps4[i],
                        bqk_sb[:, 0:1],
                    )
                # first k chunk + first two v tiles bridge the pool
                # boundary out of drained pass-1 slots
                ps = p1ps.tile([128, 512], F32, tag="g", name="kb0")
                for ko in range(KO):
                    nc.tensor.matmul(
                        ps,
                        w_sb[:, ko, 256:384],
                        hT[:, ko, 0:512],
                        start=(ko == 0), stop=(ko == KO - 1),
                    )
                nc.vector.tensor_scalar_add(
                    qk_sb[:, 2, 0:512], ps, bqk_sb[:, 2:3],
                )
                for st in range(2):
                    ps = p1ps.tile([128, 512], F32, tag="g", name=f"vb{st}")
                    for ko in range(KO):
                        nc.tensor.matmul(
                            ps[:, 0:256],
                            hT[:, ko, 128 * st:128 * st + 128],
                            w_sb[:, ko, 512:768],
                            start=(ko == 0), stop=(ko == KO - 1),
                        )
                    nc.vector.tensor_add(
                        out=v_sb[:, st, :, 0:64],
                        in0=ps[:, 0:256].rearrange("p (h d) -> p h d", d=64),
                        in1=bv_sb.rearrange("p (h d) -> p h d", d=64),
                    )

            # ================= attention + everything else =================
            # pool order fixes which freed qk-round banks each tag aliases:
            # ppr first so the w0 v-rounds can start as soon as the first
            # qk pass drains.  av/tp are single-bank tiles whose 65/128-col
            # sub-slices rotate manually: depth-4 pipelining in one bank
            # each (PSUM allocation is bank-granular, so separate tiles
            # would blow the 8-bank budget).
            with tc.tile_pool(name="ppr", bufs=2, space="PSUM") as ppr, \
                 tc.tile_pool(name="psc", bufs=1, space="PSUM") as psc, \
                 tc.tile_pool(name="pav", bufs=1, space="PSUM") as pav:

                # scores ring: 5 banks of 512 fp32 columns; pieces take 1-2
                # adjacent slots, giving pipeline depth ~2.5-5 between the
                # score matmuls and their exp drains.
                sc_big = psc.tile([128, 2560], F32)
                # AV accumulators (4 x 65 fp32) and transpose outputs
                # (3 x 128 bf16) share one bank
                avtp = pav.tile([128, 452], F32)
                av_big = avtp[:, 0:260]
                tp_big = avtp[:, 260:452].bitcast(BF)
                stg_by = {}

                sc_ptr = [0]

                def emit_scores(lh, t):
                    """score pieces + exp into at bufs, then diag mask."""
                    bp = 64 * (lh % 2)
                    qn, kn = lh // 2, 2 + lh // 2
                    at_sb = at_by[lh]
                    gs = 128 * t
                    while gs < S:
                        w = min(1024, S - gs)
                        nsl = (w + 511) // 512
                        if sc_ptr[0] + nsl > 5:
                            sc_ptr[0] = 0
                        sc = sc_big[:, 512 * sc_ptr[0]:512 * sc_ptr[0] + w]
                        sc_ptr[0] += nsl
                        o = 0
                        while o < w:
                            n = min(512, w - o)
                            nc.tensor.matmul(
                                sc[:, o:o + n],
                                qk_sb[bp:bp + 64, kn, 128 * t:128 * t + 128],
                                qk_sb[bp:bp + 64, qn, gs + o:gs + o + n],
                                start=True, stop=True,
                            )
                            o += n
                        nc.scalar.activation(
                            at_sb[:, OFF[t] + gs - 128 * t:
                                  OFF[t] + gs - 128 * t + w],
                            sc[:, :w],
                            mybir.ActivationFunctionType.Exp,
                            bias=lm_sb[:, lh, t:t + 1], scale=0.125,
                        )
                        gs += w
                    # causal 0/1 mask on the diagonal block (GPSIMD,
                    # all-SBUF, never gates ScalarE).
                    nc.gpsimd.tensor_mul(
                        out=at_by[lh][:, OFF[t]:OFF[t] + 128],
                        in0=at_by[lh][:, OFF[t]:OFF[t] + 128],
                        in1=dm_sb,
                    )

                av_rr = [0]
                tp_rr = [0]

                def emit_av(lh, t):
                    """flipped AV for q-tile t + per-partition normalize."""
                    at_sb = at_by[lh]
                    c = av_rr[0] % 4
                    av_rr[0] += 1
                    av = av_big[:, 65 * c:65 * c + 65]
                    for u in range(t + 1):
                        nc.tensor.matmul(
                            av,
                            at_sb[:, OFF[u] + 128 * (t - u):
                                  OFF[u] + 128 * (t - u) + 128],
                            v_sb[:, u, lh, :],
                            start=(u == 0), stop=(u == t),
                        )
                    rec = recp.tile([128, 1], F32, tag="rec")
                    nc.vector.reciprocal(rec, av[:, 64:65])
                    if t >= 13:
                        nc.scalar.mul(
                            stg_by[lh // 2][:, t, lh % 2, :], av[:, 0:64],
                            rec,
                        )
                    else:
                        nc.vector.tensor_scalar_mul(
                            stg_by[lh // 2][:, t, lh % 2, :], av[:, 0:64],
                            rec,
                        )

                def emit_transpose(j, st, on_act=False):
                    """O' [q,(e,d)] block -> ao2 [(e,d), q] via PE."""
                    c = tp_rr[0] % 3
                    tp_rr[0] += 1
                    tp = tp_big[:, 128 * c:128 * c + 128]
                    nc.tensor.matmul(
                        tp, stg_by[j][:, st, :, :], id_sb,
                        start=True, stop=True, is_transpose=True,
                    )
                    if on_act:
                        nc.scalar.copy(ao2[:, j, 128 * st:128 * st + 128], tp)
                    else:
                        nc.vector.tensor_copy(
                            ao2[:, j, 128 * st:128 * st + 128], tp,
                        )

                def emit_cproj(st, ec, on_act=False):
                    pr = ppr.tile([128, 512], F32, tag="pr")
                    for jj in range(2):
                        nc.tensor.matmul(
                            pr,
                            ao2[:, jj, 128 * st:128 * st + 128],
                            pw_sb[:, jj, 512 * ec:512 * ec + 512],
                            start=(jj == 0), stop=(jj == 1),
                        )
                    o_sb = outp.tile([128, 512], BF, tag="osb")
                    if on_act:
                        nc.scalar.copy(o_sb, pr)
                    else:
                        nc.vector.tensor_copy(o_sb, pr)
                    nc.sync.dma_start(
                        out[128 * st:128 * st + 128,
                            512 * ec:512 * ec + 512],
                        o_sb,
                    )

                def emit_vround(st):
                    """v natural: out[s-tile, (h,d)] = hidden @ wv."""
                    ps = ppr.tile([128, 512], F32, tag="pr", name=f"v{st}")
                    for ko in range(KO):
                        nc.tensor.matmul(
                            ps[:, 0:256],
                            hT[:, ko, 128 * st:128 * st + 128],
                            w_sb[:, ko, 512:768],
                            start=(ko == 0), stop=(ko == KO - 1),
                        )
                    nc.vector.tensor_add(
                        out=v_sb[:, st, :, 0:64],
                        in0=ps[:, 0:256].rearrange("p (h d) -> p h d", d=64),
                        in1=bv_sb.rearrange("p (h d) -> p h d", d=64),
                    )

                def emit_proj(nt, sc4):
                    """one deferred qk projection chunk (ko-inner)."""
                    ps = ppr.tile([128, 512], F32, tag="pr",
                                  name=f"qp{nt}{sc4}")
                    for ko in range(KO):
                        nc.tensor.matmul(
                            ps,
                            w_sb[:, ko, 128 * nt:128 * nt + 128],
                            hT[:, ko, 512 * sc4:512 * sc4 + 512],
                            start=(ko == 0), stop=(ko == KO - 1),
                        )
                    nc.vector.tensor_scalar_add(
                        qk_sb[:, nt, 512 * sc4:512 * sc4 + 512], ps,
                        bqk_sb[:, nt:nt + 1],
                    )

                at_by = {}

                def new_at(lh):
                    if lh == 2:
                        at_by[2] = at2_view
                    elif lh == 3:
                        at_by[3] = at3_sb
                    else:
                        at_by[lh] = atp.tile([128, ATW], BF, tag="at",
                                             name=f"at{lh}")
                    if lh // 2 not in stg_by:
                        stg_by[lh // 2] = stgp.tile([128, NT, 2, 64], BF,
                                                    tag="stg",
                                                    name=f"stg{lh // 2}")

                # ============ global greedy PE/ACT budgeter ============
                # One global stream of score rounds (heads 0+1 interleaved,
                # then 3+2 staggered).  AV / transpose / c_proj chase
                # dependency pointers.  A tiny time model runs alongside:
                # pe_t = PE busy time emitted so far (the emission order IS
                # the PE execution order), act_end = when ScalarE finishes
                # everything queued so far.  Fillers (v rounds, deferred
                # projections) drain whenever ScalarE's backlog covers
                # their PE time, so ScalarE never starves; hard deadlines
                # protect correctness.  Late c_proj/transpose drain copies
                # ride ScalarE, which is idle once the exps run out.
                clk = {"pe": 10200.0, "act": 0.0}

                def c_scores(lh, t):
                    emit_scores(lh, t)
                    w = S - 128 * t
                    clk["pe"] += w * 0.4167
                    clk["act"] = max(clk["act"], clk["pe"]) \
                        + w * 0.833 + 200.0 * ((w + 1023) // 1024)

                def c_av(lh, t):
                    emit_av(lh, t)
                    clk["pe"] += 27.1 * (t + 1)

                def c_T(j, st):
                    late = j == 1 and st >= 12
                    emit_transpose(j, st, on_act=late)
                    clk["pe"] += 53.0
                    if late:
                        clk["act"] = max(clk["act"], clk["pe"]) + 350.0

                def c_cp(st):
                    late = st >= 10
                    emit_cproj(st, 0, on_act=False)
                    emit_cproj(st, 1, on_act=late)
                    clk["pe"] += 854.0
                    if late:
                        clk["act"] = max(clk["act"], clk["pe"]) + 610.0

                # ---- job list ----
                jobs = []
                for t in range(NT):
                    jobs.append((0, t))
                    jobs.append((1, t))
                for t in range(NT):
                    jobs.append((3, t))
                    if t >= 4:
                        jobs.append((2, t - 4))
                for u in range(NT - 4, NT):
                    jobs.append((2, u))
                ji = {}
                for i, j in enumerate(jobs):
                    ji[j] = i
                J_B = ji[(3, 0)]
                J_H2 = ji[(2, 0)]

                # ---- filler queue: (deadline job idx, emit fn).
                # Deadlines are per-consumer: k chunk c before scores
                # (0/1, 4c); v(u) before the first AV that reads it and in
                # any case before head 2's first exp overwrites hT; the
                # nt1 chunks + nt3 chunk 0 before head 3 starts, the other
                # nt3 chunks before head 3's tile 4c. ----
                vd = [1]    # v tiles 0..vd[0] exist (v0/v1 from prologue)

                def emit_v_count(u):
                    emit_vround(u)
                    vd[0] = u

                fillers = []
                for c in range(1, 4):
                    fillers.append((8 * c,
                                    (lambda c=c: emit_proj(2, c), 1707.0)))
                for g in range(4):
                    fillers.append((J_B - 1,
                                    (lambda g=g: emit_proj(1, g), 1707.0)))
                fillers.append((J_B - 1, (lambda: emit_proj(3, 0), 1707.0)))
                for c in range(1, 4):
                    fillers.append((ji[(3, 4 * c)] - 1,
                                    (lambda c=c: emit_proj(3, c), 1707.0)))
                # v rounds gate only head 2 (hT overlay WAR) and the
                # head-0/1 AV chase, which is capped by vd
                for u in range(2, NT):
                    fillers.append((min(J_B + u, J_H2 - 1),
                                    (lambda u=u: emit_v_count(u), 853.0)))
                fillers.sort(key=lambda x: x[0])
                fillers = fillers[::-1]   # pop from the end

                scored = {}
                avd = {0: -1, 1: -1, 2: -1, 3: -1}
                t0d, t1d, cpd = [-1], [-1], [-1]

                def av_tgt(s):
                    return max(min(s - 1, 2), s - 3 if s < 11 else s - 2)

                def chase(budget_hungry):
                    # mandatory dependency-paced work, at most a step each
                    for lh in (0, 1, 3, 2):
                        if lh in scored:
                            tgt = min(av_tgt(scored[lh]), NT - 1)
                            if lh < 2:
                                tgt = min(tgt, vd[0])
                            while avd[lh] < tgt:
                                avd[lh] += 1
                                c_av(lh, avd[lh])
                    if t0d[0] < min(avd[0], avd[1]) - 1:
                        t0d[0] += 1
                        c_T(0, t0d[0])
                    if t1d[0] < min(avd[3], avd[2]) - 1:
                        t1d[0] += 1
                        c_T(1, t1d[0])
                    ncp = 2 if budget_hungry else 1
                    for _ in range(ncp):
                        if cpd[0] < min(t0d[0], t1d[0]) - 1:
                            cpd[0] += 1
                            c_cp(cpd[0])

                new_at(0)
                new_at(1)
                for i, (lh, t) in enumerate(jobs):
                    if (lh, t) == (2, 0):
                        new_at(2)
                    if i == J_B:
                        new_at(3)
                    c_scores(lh, t)
                    scored[lh] = max(scored.get(lh, -1), t)
                    chase(clk["act"] > clk["pe"] + 2500.0)
                    # deadline-forced fillers, then backlog-driven drain:
                    # a filler is free when ScalarE still has that much
                    # queued work to chew through
                    while fillers and fillers[-1][0] <= i:
                        _, (fn, pe) = fillers.pop()
                        fn()
                        clk["pe"] += pe
                    if fillers and i >= J_B:
                        # in window B ScalarE runs a real backlog the clock
                        # model can't see (it idled through the boundary
                        # dump): drain one filler per job unconditionally
                        _, (fn, pe) = fillers.pop()
                        fn()
                        clk["pe"] += pe
                    while fillers and \
                            clk["act"] - clk["pe"] > fillers[-1][1][1] + 300:
                        _, (fn, pe) = fillers.pop()
                        fn()
                        clk["pe"] += pe
                # ---- tail: drain all remaining pointer work ----
                for _ in range(3 * NT):
                    before = (tuple(avd.values()), t0d[0], t1d[0], cpd[0])
                    for lh in (0, 1, 3, 2):
                        if lh in scored and avd[lh] < NT - 1:
                            avd[lh] += 1
                            c_av(lh, avd[lh])
                    if t0d[0] < min(avd[0], avd[1]) - 1 or \
                       (min(avd[0], avd[1]) == NT - 1 and t0d[0] < NT - 1):
                        t0d[0] += 1
                        c_T(0, t0d[0])
                    if t1d[0] < min(avd[3], avd[2]) - 1 or \
                       (min(avd[3], avd[2]) == NT - 1 and t1d[0] < NT - 1):
                        t1d[0] += 1
                        c_T(1, t1d[0])
                    lim = min(t0d[0], t1d[0])
                    lim = lim if lim < NT - 1 else NT - 1
                    for _ in range(2):
                        if cpd[0] < lim - (0 if lim == NT - 1 else 1):
                            cpd[0] += 1
                            c_cp(cpd[0])
                    after = (tuple(avd.values()), t0d[0], t1d[0], cpd[0])
                    if before == after and cpd[0] >= NT - 1:
                        break
                while cpd[0] < NT - 1:
                    cpd[0] += 1
                    c_cp(cpd[0])
    return nc


_NC = None


def _get_nc():
    global _NC
    if _NC is None:
        _NC = build_program()
    return _NC


# ---------------------------------------------------------------- host prep

def make_in_maps(hidden_states, c_attn_w, c_attn_b, c_proj_w):
    first_end = S // 3
    second_end = 2 * S // 3
    pos = np.arange(S)
    regions = [pos < first_end,
               (pos >= first_end) & (pos < second_end),
               pos >= second_end]
    mult = np.ones((H, S), dtype=np.float64)
    for h, r in HEAD_REGION.items():
        mult[h] = 1.0 + (FOCUS - 1.0) * regions[r].astype(np.float64)
    logm = np.log(mult).astype(np.float32)  # [H, S]

    p = np.arange(128)[:, None]
    j = np.arange(128)[None, :]
    diag = (j >= p).astype(np.float32)  # 0/1 keep-mask, applied post-exp
    iden = (j == p).astype(np.float32)

    bf = ml_dtypes.bfloat16
    in_maps = []
    for c in range(NCORES):
        b, g = divmod(c, GROUPS)
        h0 = HPC * g
        cs = slice(256 * g, 256 * g + 256)
        w_qkv = np.concatenate(
            [c_attn_w[:, cs], c_attn_w[:, 1024:2048][:, cs],
             c_attn_w[:, 2048:3072][:, cs]], axis=1,
        ).astype(bf)
        bqk_h = np.concatenate(
            [c_attn_b[cs], c_attn_b[1024:2048][cs]]
        ).reshape(4, 128).T.copy().astype(np.float32)
        bv = np.broadcast_to(
            c_attn_b[2048:3072][cs], (128, 256)
        ).astype(np.float32).copy()
        # pw2[p, j, e]: head pair j=(2j, 2j+1); p<64 -> head 2j row p,
        # p>=64 -> head 2j+1 row p-64  (matches ao2 partition interleave)
        pw = c_proj_w[64 * h0:64 * h0 + 256, :].reshape(2, 128, D)
        pw = np.ascontiguousarray(pw.transpose(1, 0, 2)).astype(bf)
        lm = logm[h0:h0 + HPC].reshape(HPC, S // 128, 128)
        lm = np.ascontiguousarray(lm.transpose(2, 0, 1)).astype(np.float32)
        in_maps.append({
            "hiddenT": np.ascontiguousarray(hidden_states[b].T).astype(bf),
            "w_qkv": w_qkv,
            "bqk": bqk_h,
            "bv_rep": bv,
            "projw": pw,
            "diag_mask": diag.astype(bf),
            "ident": iden.astype(bf),
            "logmult": lm,
        })
    return in_maps


def run_cores(in_maps, trace=False, **kw):
    from concourse.bass_utils import run_bass_kernel_spmd
    nc = _get_nc()
    return run_bass_kernel_spmd(nc, in_maps, core_ids=list(range(NCORES)),
                                trace=trace, **kw)


def kernel(hidden_states, c_attn_w, c_attn_b, c_proj_w, c_proj_b):
    hidden_states = np.asarray(hidden_states, dtype=np.float32)
    c_attn_w = np.asarray(c_attn_w, dtype=np.float32)
    c_attn_b = np.asarray(c_attn_b, dtype=np.float32)
    c_proj_w = np.asarray(c_proj_w, dtype=np.float32)
    c_proj_b = np.asarray(c_proj_b, dtype=np.float32)

    in_maps = make_in_maps(hidden_states, c_attn_w, c_attn_b, c_proj_w)
    res = run_cores(in_maps)
    out = np.zeros((B, S, D), dtype=np.float32)
    for c in range(NCORES):
        out[c // GROUPS] += np.asarray(res.results[c]["out"],
                                       dtype=np.float32)
    out += c_proj_b[None, None, :]
    return out


# revision 50
# speedup vs baseline: 1.3815x; 1.3815x over previous
"""DivergentAttention Trainium2 kernel (8 NeuronCores, Bass/Tile).

Problem: GPT-2 style causal self-attention (B=2, S=2048, D=1024, H=16,
hd=64) where heads 0/1/2 re-weight their attention toward a token region
(first/middle/last third of the sequence) with factor 1.6 and renormalize.

Key identity: softmax(s)*m / sum(softmax(s)*m) == softmax(s + log m), so the
per-head region reweight folds into an additive per-(head, key-position)
bias on the scores -- no second normalization pass needed.  Scores are small
(|s|<~5) so the max-subtraction pass is skipped entirely.

Sharding: tensor-parallel over (batch, head-group): core c handles batch
c//4 and heads [4*(c%4), 4*(c%4)+4).  Each core computes the QKV projection
for its 4 heads, full causal attention, and its partial c_proj; the host
sums the 8 partials (fp32) and adds c_proj_b.

All matmul inputs are bf16 (error budget allows it; bf16 runs the PE at one
row per output column for every width, unlike fp32r which needs N>=256).

Attention structure per head (the big change vs the v1 kernel):
  - scoresT [sk-tile 128, q] = kT.T @ qT as before, exp'd (ScalarE, scale
    1/8, bias log mult) into a per-head PERSISTENT attn buffer at_sb
    [128, 17408] bf16 holding the whole causal triangle.
  - AV is FLIPPED: out[q-tile 128, 65] = attnT(stationary).T @ v_aug --
    65 output columns per (q-tile, sk-tile) pair instead of 512-wide
    rows, halving AV PE time.  v_aug column 64 is ones, so column 64 of
    the output is the softmax denominator per q ROW -- normalization
    becomes a per-partition scalar multiply (DVE reciprocal + mult), no
    partition-broadcast DMA bounce.
  - The normalized per-head output O' [q, 64] bf16 is transposed back to
    [64*(h%2)+d, q] for c_proj with a PE transpose (identity moving
    operand), 128 rows per q-tile.
  - c_proj: two K=128 matmuls per output tile against pw2 [128, 2, D].

Window schedule (PE-balance: exp on ScalarE is the per-head pacing limit,
so PE filler work is spread into the exp-bound windows):
  w0: head 0 + one v-projection tile per round (pr-tag PSUM).
  w1: head 1 + the deferred qk projection round for heads 2/3 (8 groups,
      rounds 0..7) + pair-0 transposes.
  w2: head 3 + head-2 score tiles 0..4 pulled ahead (rounds 11..15).
  w3: head 2 (scores 5..15) + pair-1 transposes + c_proj, software-
      pipelined (transpose at round t-2, c_proj at round t-3) so the PE
      never waits on the DVE/ScalarE drains.
"""

import numpy as np
import ml_dtypes

import concourse.bass as bass
import concourse.tile as tile
from concourse import mybir
from concourse import bass_utils, bass2jax

# ---------------------------------------------------------------- constants
B, S, D, H, HD = 2, 2048, 1024, 16, 64
NCORES = 8
HPC = 4              # heads per core
GROUPS = 4           # head groups
FOCUS = 1.6
HEAD_REGION = {0: 0, 1: 1, 2: 2}
NT = S // 128        # 16 sk/q tiles
KO = D // 128        # 8 contraction chunks
BF = mybir.dt.bfloat16
F32 = mybir.dt.float32

# column offset of tile t's rows inside the per-head attn buffer
OFF = [0] * (NT + 1)
for _t in range(NT):
    OFF[_t + 1] = OFF[_t] + (S - 128 * _t)
ATW = OFF[NT]        # 17408

# ------------------------------------------------- walrus multi-wait fixup
# This container's walrus accepts only ONE sync-wait per TPB instruction,
# but Tile attaches one wait per dependency proc.  Rewrite the BIR JSON just
# before walrus: hoist all-but-one wait of a multi-wait instruction onto
# standalone same-engine NoOps inserted immediately before it (same-engine
# program order is preserved, so semantics are unchanged).
try:
    import orjson as _json
except ImportError:  # pragma: no cover
    import json as _json

_orig_compile_bir_kernel = bass_utils.compile_bir_kernel
_wfix_counter = [0]


def _fix_bir(bir_json):
    d = _json.loads(bir_json)
    changed = False
    for fn in d.get("functions", []):
        for blk in fn.get("blocks", []):
            out = []
            for inst in blk.get("instructions", []):
                si = inst.get("sync_info")
                if si:
                    waits = si.get("on_wait") or []
                    if len(waits) > 1:
                        changed = True
                        for w in waits[:-1]:
                            _wfix_counter[0] += 1
                            nop = {
                                "engine": inst["engine"],
                                "ins": [],
                                "name": f"I-wfix-{_wfix_counter[0]}",
                                "opcode": "NoOp",
                                "outs": [],
                                "sync_info": {"on_update": [], "on_wait": [w]},
                            }
                            if "debug" in inst:
                                nop["debug"] = inst["debug"]
                            out.append(nop)
                        si["on_wait"] = waits[-1:]
                out.append(inst)
            blk["instructions"] = out
    return _json.dumps(d) if changed else bir_json


def _patched_compile_bir_kernel(bir_json, tmpdir, neff_name="file.neff"):
    return _orig_compile_bir_kernel(_fix_bir(bir_json), tmpdir, neff_name=neff_name)


def _install_waitfix():
    bass_utils.compile_bir_kernel = _patched_compile_bir_kernel
    bass2jax.compile_bir_kernel = _patched_compile_bir_kernel


_install_waitfix()

# ---------------------------------------------------------------- program


def build_program():
    """One SPMD Bass program; per-core differences come in via inputs."""
    nc = bass.Bass()

    hiddenT = nc.dram_tensor("hiddenT", [D, S], BF, kind="ExternalInput")
    w_qkv = nc.dram_tensor("w_qkv", [D, 768], BF, kind="ExternalInput")
    bqk = nc.dram_tensor("bqk", [128, 4], F32, kind="ExternalInput")
    bv_rep = nc.dram_tensor("bv_rep", [128, 256], F32, kind="ExternalInput")
    projw = nc.dram_tensor("projw", [128, 2, D], BF, kind="ExternalInput")
    diag_mask = nc.dram_tensor("diag_mask", [128, 128], BF, kind="ExternalInput")
    ident = nc.dram_tensor("ident", [128, 128], BF, kind="ExternalInput")
    logmult = nc.dram_tensor("logmult", [128, HPC, NT], F32, kind="ExternalInput")
    out = nc.dram_tensor("out", [S, D], BF, kind="ExternalOutput")

    with tile.TileContext(nc) as tc:
        with tc.tile_pool(name="persist", bufs=1) as persist, \
             tc.tile_pool(name="atp", bufs=2) as atp, \
             tc.tile_pool(name="stgp", bufs=2) as stgp, \
             tc.tile_pool(name="recp", bufs=4) as recp, \
             tc.tile_pool(name="outp", bufs=4) as outp:

            # ---- persistent SBUF ----
            # hT and w_sb die after window 1 (last v-round / qk-r1 reads);
            # head 2's attention buffer manually reuses their bytes via
            # overlapping views of one scratch tile (Tile's AP-range WAR
            # tracking orders the reuse).
            scratch = persist.tile([128, 23552], BF)      # 5.75 MB
            hT = scratch[:, 0:KO * S].rearrange("p (ko s) -> p ko s", s=S)
            w_sb = scratch[:, KO * S:KO * S + KO * 768].rearrange(
                "p (ko n) -> p ko n", n=768)
            at2_view = scratch[:, 0:ATW]
            at3_sb = persist.tile([128, ATW], BF)         # head 3, no WAR
            qk_sb = persist.tile([128, 4, S], BF)         # 2 MB
            v_sb = persist.tile([128, NT, HPC, 65], BF)   # ~1 MB
            ao2 = persist.tile([128, 2, S], BF)           # attn_outT, 1 MB
            bqk_sb = persist.tile([128, 4], F32)
            bv_sb = persist.tile([128, 256], F32)
            pw_sb = persist.tile([128, 2, D], BF)         # 0.5 MB
            dm_sb = persist.tile([128, 128], BF)
            id_sb = persist.tile([128, 128], BF)
            lm_sb = persist.tile([128, HPC, NT], F32)

            nc.sync.dma_start(bqk_sb, bqk[:, :])
            nc.vector.memset(v_sb[:, :, :, 64:65], 1.0)

            # input loads: hT alternates the SP and ACT HWDGE queues (ACT
            # is idle until the first exp), w + small tensors ride the
            # GPSIMD SWDGE queue.
            hT_src = hiddenT.rearrange("(ko p) s -> p ko s", p=128)
            w_src = w_qkv.rearrange("(ko p) n -> p ko n", p=128)
            # first w slice + first hT chunk in small pieces so the first
            # matmul can fire as early as possible
            nc.sync.dma_start(w_sb[:, 0, 0:128], w_src[:, 0, 0:128])
            for pc in range(4):
                nc.sync.dma_start(hT[:, 0, 512 * pc:512 * pc + 512],
                                  hT_src[:, 0, 512 * pc:512 * pc + 512])
            nc.gpsimd.dma_start(w_sb[:, 0, 128:768], w_src[:, 0, 128:768])
            for ko in range(1, KO):
                q = nc.sync if ko % 2 == 0 else nc.scalar
                q.dma_start(hT[:, ko, :], hT_src[:, ko, :])
                nc.gpsimd.dma_start(w_sb[:, ko, :], w_src[:, ko, :])
                if ko == 1:
                    nc.sync.dma_start(bv_sb, bv_rep[:, :])
                    nc.gpsimd.dma_start(dm_sb, diag_mask[:, :])
                    nc.gpsimd.dma_start(id_sb, ident[:, :])
                    nc.gpsimd.dma_start(lm_sb, logmult[:, :, :])
                    nc.gpsimd.dma_start(pw_sb, projw[:, :, :])

            # ========== qk projection round 0 (heads 0/1: nt 0 and 2) =====
            # ko (contraction) outer with 4 resident PSUM groups per pass;
            # two passes so the first pass's banks drain (DVE) while the
            # second computes, letting the attention pools start without
            # waiting for the full round to drain.
            with tc.tile_pool(name="p1ps", bufs=8, space="PSUM") as p1ps:
                # prologue computes ONLY q of heads 0/1 (nt 0): scores for
                # k-tile t need just one 512-column k chunk, so the k
                # projection is deferred into window 0 as PE filler and
                # the first exp fires ~7us earlier.
                ps4 = [p1ps.tile([128, 512], F32, tag="g", name=f"q0{i}")
                       for i in range(4)]
                for ko in range(KO):
                    for i in range(4):
                        nc.tensor.matmul(
                            ps4[i],
                            w_sb[:, ko, 0:128],
                            hT[:, ko, 512 * i:512 * i + 512],
                            start=(ko == 0), stop=(ko == KO - 1),
                        )
                for i in range(4):
                    nc.vector.tensor_scalar_add(
                        qk_sb[:, 0, 512 * i:512 * i + 512], ps4[i],
                        bqk_sb[:, 0:1],
                    )
                # first k chunk + first two v tiles bridge the pool
                # boundary out of drained pass-1 slots
                ps = p1ps.tile([128, 512], F32, tag="g", name="kb0")
                for ko in range(KO):
                    nc.tensor.matmul(
                        ps,
                        w_sb[:, ko, 256:384],
                        hT[:, ko, 0:512],
                        start=(ko == 0), stop=(ko == KO - 1),
                    )
                nc.vector.tensor_scalar_add(
                    qk_sb[:, 2, 0:512], ps, bqk_sb[:, 2:3],
                )
                for st in range(2):
                    ps = p1ps.tile([128, 512], F32, tag="g", name=f"vb{st}")
                    for ko in range(KO):
                        nc.tensor.matmul(
                            ps[:, 0:256],
                            hT[:, ko, 128 * st:128 * st + 128],
                            w_sb[:, ko, 512:768],
                            start=(ko == 0), stop=(ko == KO - 1),
                        )
                    nc.vector.tensor_add(
                        out=v_sb[:, st, :, 0:64],
                        in0=ps[:, 0:256].rearrange("p (h d) -> p h d", d=64),
                        in1=bv_sb.rearrange("p (h d) -> p h d", d=64),
                    )

            # ================= attention + everything else =================
            # pool order fixes which freed qk-round banks each tag aliases:
            # ppr first so the w0 v-rounds can start as soon as the first
            # qk pass drains.  av/tp are single-bank tiles whose 65/128-col
            # sub-slices rotate manually: depth-4 pipelining in one bank
            # each (PSUM allocation is bank-granular, so separate tiles
            # would blow the 8-bank budget).
            with tc.tile_pool(name="ppr", bufs=2, space="PSUM") as ppr, \
                 tc.tile_pool(name="psc", bufs=2, space="PSUM") as psc, \
                 tc.tile_pool(name="pav", bufs=1, space="PSUM") as pav, \
                 tc.tile_pool(name="ptp", bufs=1, space="PSUM") as ptp:

                av_big = pav.tile([128, 260], F32)
                tp_big = ptp.tile([128, 512], BF)
                stg_by = {}

                def emit_scores(lh, t):
                    """score pieces + exp into at bufs, then diag mask."""
                    bp = 64 * (lh % 2)
                    qn, kn = lh // 2, 2 + lh // 2
                    at_sb = at_by[lh]
                    gs = 128 * t
                    while gs < S:
                        w = min(1024, S - gs)
                        sc = psc.tile([128, 1024], F32, tag="sc")
                        o = 0
                        while o < w:
                            n = min(512, w - o)
                            nc.tensor.matmul(
                                sc[:, o:o + n],
                                qk_sb[bp:bp + 64, kn, 128 * t:128 * t + 128],
                                qk_sb[bp:bp + 64, qn, gs + o:gs + o + n],
                                start=True, stop=True,
                            )
                            o += n
                        nc.scalar.activation(
                            at_sb[:, OFF[t] + gs - 128 * t:
                                  OFF[t] + gs - 128 * t + w],
                            sc[:, :w],
                            mybir.ActivationFunctionType.Exp,
                            bias=lm_sb[:, lh, t:t + 1], scale=0.125,
                        )
                        gs += w
                    # causal 0/1 mask on the diagonal block (GPSIMD,
                    # all-SBUF, never gates ScalarE).
                    nc.gpsimd.tensor_mul(
                        out=at_by[lh][:, OFF[t]:OFF[t] + 128],
                        in0=at_by[lh][:, OFF[t]:OFF[t] + 128],
                        in1=dm_sb,
                    )

                av_rr = [0]
                tp_rr = [0]

                def emit_av(lh, t):
                    """flipped AV for q-tile t + per-partition normalize."""
                    at_sb = at_by[lh]
                    c = av_rr[0] % 4
                    av_rr[0] += 1
                    av = av_big[:, 65 * c:65 * c + 65]
                    for u in range(t + 1):
                        nc.tensor.matmul(
                            av,
                            at_sb[:, OFF[u] + 128 * (t - u):
                                  OFF[u] + 128 * (t - u) + 128],
                            v_sb[:, u, lh, :],
                            start=(u == 0), stop=(u == t),
                        )
                    rec = recp.tile([128, 1], F32, tag="rec")
                    nc.vector.reciprocal(rec, av[:, 64:65])
                    if t >= 13:
                        nc.scalar.mul(
                            stg_by[lh // 2][:, t, lh % 2, :], av[:, 0:64],
                            rec,
                        )
                    else:
                        nc.vector.tensor_scalar_mul(
                            stg_by[lh // 2][:, t, lh % 2, :], av[:, 0:64],
                            rec,
                        )

                def emit_transpose(j, st, on_act=False):
                    """O' [q,(e,d)] block -> ao2 [(e,d), q] via PE."""
                    c = tp_rr[0] % 4
                    tp_rr[0] += 1
                    tp = tp_big[:, 128 * c:128 * c + 128]
                    nc.tensor.matmul(
                        tp, stg_by[j][:, st, :, :], id_sb,
                        start=True, stop=True, is_transpose=True,
                    )
                    if on_act:
                        nc.scalar.copy(ao2[:, j, 128 * st:128 * st + 128], tp)
                    else:
                        nc.vector.tensor_copy(
                            ao2[:, j, 128 * st:128 * st + 128], tp,
                        )

                def emit_cproj(st, ec, on_act=False):
                    pr = ppr.tile([128, 512], F32, tag="pr")
                    for jj in range(2):
                        nc.tensor.matmul(
                            pr,
                            ao2[:, jj, 128 * st:128 * st + 128],
                            pw_sb[:, jj, 512 * ec:512 * ec + 512],
                            start=(jj == 0), stop=(jj == 1),
                        )
                    o_sb = outp.tile([128, 512], BF, tag="osb")
                    if on_act:
                        nc.scalar.copy(o_sb, pr)
                    else:
                        nc.vector.tensor_copy(o_sb, pr)
                    nc.sync.dma_start(
                        out[128 * st:128 * st + 128,
                            512 * ec:512 * ec + 512],
                        o_sb,
                    )

                def emit_vround(st):
                    """v natural: out[s-tile, (h,d)] = hidden @ wv."""
                    ps = ppr.tile([128, 512], F32, tag="pr", name=f"v{st}")
                    for ko in range(KO):
                        nc.tensor.matmul(
                            ps[:, 0:256],
                            hT[:, ko, 128 * st:128 * st + 128],
                            w_sb[:, ko, 512:768],
                            start=(ko == 0), stop=(ko == KO - 1),
                        )
                    nc.vector.tensor_add(
                        out=v_sb[:, st, :, 0:64],
                        in0=ps[:, 0:256].rearrange("p (h d) -> p h d", d=64),
                        in1=bv_sb.rearrange("p (h d) -> p h d", d=64),
                    )

                def emit_proj(nt, sc4):
                    """one deferred qk projection chunk (ko-inner)."""
                    ps = ppr.tile([128, 512], F32, tag="pr",
                                  name=f"qp{nt}{sc4}")
                    for ko in range(KO):
                        nc.tensor.matmul(
                            ps,
                            w_sb[:, ko, 128 * nt:128 * nt + 128],
                            hT[:, ko, 512 * sc4:512 * sc4 + 512],
                            start=(ko == 0), stop=(ko == KO - 1),
                        )
                    nc.vector.tensor_scalar_add(
                        qk_sb[:, nt, 512 * sc4:512 * sc4 + 512], ps,
                        bqk_sb[:, nt:nt + 1],
                    )

                at_by = {}

                def new_at(lh):
                    if lh == 2:
                        at_by[2] = at2_view
                    elif lh == 3:
                        at_by[3] = at3_sb
                    else:
                        at_by[lh] = atp.tile([128, ATW], BF, tag="at",
                                             name=f"at{lh}")
                    if lh // 2 not in stg_by:
                        stg_by[lh // 2] = stgp.tile([128, NT, 2, 64], BF,
                                                    tag="stg",
                                                    name=f"stg{lh // 2}")

                # ============ global greedy PE/ACT budgeter ============
                # One global stream of score rounds (heads 0+1 interleaved,
                # then 3+2 staggered).  AV / transpose / c_proj chase
                # dependency pointers.  A tiny time model runs alongside:
                # pe_t = PE busy time emitted so far (the emission order IS
                # the PE execution order), act_end = when ScalarE finishes
                # everything queued so far.  Fillers (v rounds, deferred
                # projections) drain whenever ScalarE's backlog covers
                # their PE time, so ScalarE never starves; hard deadlines
                # protect correctness.  Late c_proj/transpose drain copies
                # ride ScalarE, which is idle once the exps run out.
                clk = {"pe": 10200.0, "act": 0.0}

                def c_scores(lh, t):
                    emit_scores(lh, t)
                    w = S - 128 * t
                    clk["pe"] += w * 0.4167
                    clk["act"] = max(clk["act"], clk["pe"]) \
                        + w * 0.833 + 200.0 * ((w + 1023) // 1024)

                def c_av(lh, t):
                    emit_av(lh, t)
                    clk["pe"] += 27.1 * (t + 1)

                def c_T(j, st):
                    late = j == 1 and st >= 12
                    emit_transpose(j, st, on_act=late)
                    clk["pe"] += 53.0
                    if late:
                        clk["act"] = max(clk["act"], clk["pe"]) + 350.0

                def c_cp(st):
                    late = st >= 10
                    emit_cproj(st, 0, on_act=False)
                    emit_cproj(st, 1, on_act=late)
                    clk["pe"] += 854.0
                    if late:
                        clk["act"] = max(clk["act"], clk["pe"]) + 610.0

                # ---- job list ----
                jobs = []
                for t in range(NT):
                    jobs.append((0, t))
                    jobs.append((1, t))
                for t in range(NT):
                    jobs.append((3, t))
                    if t >= 4:
                        jobs.append((2, t - 4))
                for u in range(NT - 4, NT):
                    jobs.append((2, u))
                ji = {}
                for i, j in enumerate(jobs):
                    ji[j] = i
                J_B = ji[(3, 0)]
                J_H2 = ji[(2, 0)]

                # ---- filler queue: (deadline job idx, emit fn).
                # Deadlines are per-consumer: k chunk c before scores
                # (0/1, 4c); v(u) before the first AV that reads it and in
                # any case before head 2's first exp overwrites hT; the
                # nt1 chunks + nt3 chunk 0 before head 3 starts, the other
                # nt3 chunks before head 3's tile 4c. ----
                vd = [1]    # v tiles 0..vd[0] exist (v0/v1 from prologue)

                def emit_v_count(u):
                    emit_vround(u)
                    vd[0] = u

                fillers = []
                for c in range(1, 4):
                    fillers.append((8 * c,
                                    (lambda c=c: emit_proj(2, c), 1707.0)))
                for g in range(4):
                    fillers.append((J_B - 1,
                                    (lambda g=g: emit_proj(1, g), 1707.0)))
                fillers.append((J_B - 1, (lambda: emit_proj(3, 0), 1707.0)))
                for c in range(1, 4):
                    fillers.append((ji[(3, 4 * c)] - 1,
                                    (lambda c=c: emit_proj(3, c), 1707.0)))
                # v rounds gate only head 2 (hT overlay WAR) and the
                # head-0/1 AV chase, which is capped by vd
                for u in range(2, NT):
                    fillers.append((min(J_B + u, J_H2 - 1),
                                    (lambda u=u: emit_v_count(u), 853.0)))
                fillers.sort(key=lambda x: x[0])
                fillers = fillers[::-1]   # pop from the end

                scored = {}
                avd = {0: -1, 1: -1, 2: -1, 3: -1}
                t0d, t1d, cpd = [-1], [-1], [-1]

                def av_tgt(s):
                    return max(min(s - 1, 2), s - 3 if s < 11 else s - 2)

                def chase(budget_hungry):
                    # mandatory dependency-paced work, at most a step each
                    for lh in (0, 1, 3, 2):
                        if lh in scored:
                            tgt = min(av_tgt(scored[lh]), NT - 1)
                            if lh < 2:
                                tgt = min(tgt, vd[0])
                            while avd[lh] < tgt:
                                avd[lh] += 1
                                c_av(lh, avd[lh])
                    if t0d[0] < min(avd[0], avd[1]) - 1:
                        t0d[0] += 1
                        c_T(0, t0d[0])
                    if t1d[0] < min(avd[3], avd[2]) - 1:
                        t1d[0] += 1
                        c_T(1, t1d[0])
                    ncp = 2 if budget_hungry else 1
                    for _ in range(ncp):
                        if cpd[0] < min(t0d[0], t1d[0]) - 1:
                            cpd[0] += 1
                            c_cp(cpd[0])

                new_at(0)
                new_at(1)
                for i, (lh, t) in enumerate(jobs):
                    if (lh, t) == (2, 0):
                        new_at(2)
                    if i == J_B:
                        new_at(3)
                    c_scores(lh, t)
                    scored[lh] = max(scored.get(lh, -1), t)
                    chase(clk["act"] > clk["pe"] + 2500.0)
                    # deadline-forced fillers, then backlog-driven drain:
                    # a filler is free when ScalarE still has that much
                    # queued work to chew through
                    while fillers and fillers[-1][0] <= i:
                        _, (fn, pe) = fillers.pop()
                        fn()
                        clk["pe"] += pe
                    if fillers and i >= J_B:
                        # in window B ScalarE runs a real backlog the clock
                        # model can't see (it idled through the boundary
                        # dump): drain one filler per job unconditionally
                        _, (fn, pe) = fillers.pop()
                        fn()
                        clk["pe"] += pe
                    while fillers and \
                            clk["act"] - clk["pe"] > fillers[-1][1][1] + 300:
                        _, (fn, pe) = fillers.pop()
                        fn()
                        clk["pe"] += pe
                # ---- tail: drain all remaining pointer work ----
                for _ in range(3 * NT):
                    before = (tuple(avd.values()), t0d[0], t1d[0], cpd[0])
                    for lh in (0, 1, 3, 2):
                        if lh in scored and avd[lh] < NT - 1:
                            avd[lh] += 1
                            c_av(lh, avd[lh])
                    if t0d[0] < min(avd[0], avd[1]) - 1 or \
                       (min(avd[0], avd[1]) == NT - 1 and t0d[0] < NT - 1):
                        t0d[0] += 1
                        c_T(0, t0d[0])
                    if t1d[0] < min(avd[3], avd[2]) - 1 or \
                       (min(avd[3], avd[2]) == NT - 1 and t1d[0] < NT - 1):
                        t1d[0] += 1
                        c_T(1, t1d[0])
                    lim = min(t0d[0], t1d[0])
                    lim = lim if lim < NT - 1 else NT - 1
                    for _ in range(2):
                        if cpd[0] < lim - (0 if lim == NT - 1 else 1):
                            cpd[0] += 1
                            c_cp(cpd[0])
                    after = (tuple(avd.values()), t0d[0], t1d[0], cpd[0])
                    if before == after and cpd[0] >= NT - 1:
                        break
                while cpd[0] < NT - 1:
                    cpd[0] += 1
                    c_cp(cpd[0])
    return nc


_NC = None


def _get_nc():
    global _NC
    if _NC is None:
        _NC = build_program()
    return _NC


# ---------------------------------------------------------------- host prep

def make_in_maps(hidden_states, c_attn_w, c_attn_b, c_proj_w):
    first_end = S // 3
    second_end = 2 * S // 3
    pos = np.arange(S)
    regions = [pos < first_end,
               (pos >= first_end) & (pos < second_end),
               pos >= second_end]
    mult = np.ones((H, S), dtype=np.float64)
    for h, r in HEAD_REGION.items():
        mult[h] = 1.0 + (FOCUS - 1.0) * regions[r].astype(np.float64)
    logm = np.log(mult).astype(np.float32)  # [H, S]

    p = np.arange(128)[:, None]
    j = np.arange(128)[None, :]
    diag = (j >= p).astype(np.float32)  # 0/1 keep-mask, applied post-exp
    iden = (j == p).astype(np.float32)

    bf = ml_dtypes.bfloat16
    in_maps = []
    for c in range(NCORES):
        b, g = divmod(c, GROUPS)
        h0 = HPC * g
        cs = slice(256 * g, 256 * g + 256)
        w_qkv = np.concatenate(
            [c_attn_w[:, cs], c_attn_w[:, 1024:2048][:, cs],
             c_attn_w[:, 2048:3072][:, cs]], axis=1,
        ).astype(bf)
        bqk_h = np.concatenate(
            [c_attn_b[cs], c_attn_b[1024:2048][cs]]
        ).reshape(4, 128).T.copy().astype(np.float32)
        bv = np.broadcast_to(
            c_attn_b[2048:3072][cs], (128, 256)
        ).astype(np.float32).copy()
        # pw2[p, j, e]: head pair j=(2j, 2j+1); p<64 -> head 2j row p,
        # p>=64 -> head 2j+1 row p-64  (matches ao2 partition interleave)
        pw = c_proj_w[64 * h0:64 * h0 + 256, :].reshape(2, 128, D)
        pw = np.ascontiguousarray(pw.transpose(1, 0, 2)).astype(bf)
        lm = logm[h0:h0 + HPC].reshape(HPC, S // 128, 128)
        lm = np.ascontiguousarray(lm.transpose(2, 0, 1)).astype(np.float32)
        in_maps.append({
            "hiddenT": np.ascontiguousarray(hidden_states[b].T).astype(bf),
            "w_qkv": w_qkv,
            "bqk": bqk_h,
            "bv_rep": bv,
            "projw": pw,
            "diag_mask": diag.astype(bf),
            "ident": iden.astype(bf),
            "logmult": lm,
        })
    return in_maps


def run_cores(in_maps, trace=False, **kw):
    from concourse.bass_utils import run_bass_kernel_spmd
    nc = _get_nc()
    return run_bass_kernel_spmd(nc, in_maps, core_ids=list(range(NCORES)),
                                trace=trace, **kw)


def kernel(hidden_states, c_attn_w, c_attn_b, c_proj_w, c_proj_b):
    hidden_states = np.asarray(hidden_states, dtype=np.float32)
    c_attn_w = np.asarray(c_attn_w, dtype=np.float32)
    c_attn_b = np.asarray(c_attn_b, dtype=np.float32)
    c_proj_w = np.asarray(c_proj_w, dtype=np.float32)
    c_proj_b = np.asarray(c_proj_b, dtype=np.float32)

    in_maps = make_in_maps(hidden_states, c_attn_w, c_attn_b, c_proj_w)
    res = run_cores(in_maps)
    out = np.zeros((B, S, D), dtype=np.float32)
    for c in range(NCORES):
        out[c // GROUPS] += np.asarray(res.results[c]["out"],
                                       dtype=np.float32)
    out += c_proj_b[None, None, :]
    return out


# revision 63
# speedup vs baseline: 1.4930x; 1.0807x over previous
"""DivergentAttention Trainium2 kernel (8 NeuronCores, Bass/Tile).

Problem: GPT-2 style causal self-attention (B=2, S=2048, D=1024, H=16,
hd=64) where heads 0/1/2 re-weight their attention toward a token region
(first/middle/last third of the sequence) with factor 1.6 and renormalize.

Key identity: softmax(s)*m / sum(softmax(s)*m) == softmax(s + log m), so the
per-head region reweight folds into an additive per-(head, key-position)
bias on the scores -- no second normalization pass needed.  Scores are small
(|s|<~5) so the max-subtraction pass is skipped entirely.

Sharding: tensor-parallel over (batch, head-group): core c handles batch
c//4 and heads [4*(c%4), 4*(c%4)+4).  Each core computes the QKV projection
for its 4 heads, full causal attention, and its partial c_proj; the host
sums the 8 partials (fp32) and adds c_proj_b.

All matmul inputs are bf16 (error budget allows it; bf16 runs the PE at one
row per output column for every width, unlike fp32r which needs N>=256).

Attention structure per head (the big change vs the v1 kernel):
  - scoresT [sk-tile 128, q] = kT.T @ qT as before, exp'd (ScalarE, scale
    1/8, bias log mult) into a per-head PERSISTENT attn buffer at_sb
    [128, 17408] bf16 holding the whole causal triangle.
  - AV is FLIPPED: out[q-tile 128, 65] = attnT(stationary).T @ v_aug --
    65 output columns per (q-tile, sk-tile) pair instead of 512-wide
    rows, halving AV PE time.  v_aug column 64 is ones, so column 64 of
    the output is the softmax denominator per q ROW -- normalization
    becomes a per-partition scalar multiply (DVE reciprocal + mult), no
    partition-broadcast DMA bounce.
  - The normalized per-head output O' [q, 64] bf16 is transposed back to
    [64*(h%2)+d, q] for c_proj with a PE transpose (identity moving
    operand), 128 rows per q-tile.
  - c_proj: two K=128 matmuls per output tile against pw2 [128, 2, D].

Window schedule (PE-balance: exp on ScalarE is the per-head pacing limit,
so PE filler work is spread into the exp-bound windows):
  w0: head 0 + one v-projection tile per round (pr-tag PSUM).
  w1: head 1 + the deferred qk projection round for heads 2/3 (8 groups,
      rounds 0..7) + pair-0 transposes.
  w2: head 3 + head-2 score tiles 0..4 pulled ahead (rounds 11..15).
  w3: head 2 (scores 5..15) + pair-1 transposes + c_proj, software-
      pipelined (transpose at round t-2, c_proj at round t-3) so the PE
      never waits on the DVE/ScalarE drains.
"""

import numpy as np
import ml_dtypes

import concourse.bass as bass
import concourse.tile as tile
from concourse import mybir
from concourse import bass_utils, bass2jax

# ---------------------------------------------------------------- constants
B, S, D, H, HD = 2, 2048, 1024, 16, 64
NCORES = 8
HPC = 4              # heads per core
GROUPS = 4           # head groups
FOCUS = 1.6
HEAD_REGION = {0: 0, 1: 1, 2: 2}
NT = S // 128        # 16 sk/q tiles
KO = D // 128        # 8 contraction chunks
BF = mybir.dt.bfloat16
F32 = mybir.dt.float32

# column offset of tile t's rows inside the per-head attn buffer
OFF = [0] * (NT + 1)
for _t in range(NT):
    OFF[_t + 1] = OFF[_t] + (S - 128 * _t)
ATW = OFF[NT]        # 17408

# ------------------------------------------------- walrus multi-wait fixup
# This container's walrus accepts only ONE sync-wait per TPB instruction,
# but Tile attaches one wait per dependency proc.  Rewrite the BIR JSON just
# before walrus: hoist all-but-one wait of a multi-wait instruction onto
# standalone same-engine NoOps inserted immediately before it (same-engine
# program order is preserved, so semantics are unchanged).
try:
    import orjson as _json
except ImportError:  # pragma: no cover
    import json as _json

_orig_compile_bir_kernel = bass_utils.compile_bir_kernel
_wfix_counter = [0]


def _fix_bir(bir_json):
    d = _json.loads(bir_json)
    changed = False
    for fn in d.get("functions", []):
        for blk in fn.get("blocks", []):
            out = []
            for inst in blk.get("instructions", []):
                si = inst.get("sync_info")
                if si:
                    waits = si.get("on_wait") or []
                    if len(waits) > 1:
                        changed = True
                        for w in waits[:-1]:
                            _wfix_counter[0] += 1
                            nop = {
                                "engine": inst["engine"],
                                "ins": [],
                                "name": f"I-wfix-{_wfix_counter[0]}",
                                "opcode": "NoOp",
                                "outs": [],
                                "sync_info": {"on_update": [], "on_wait": [w]},
                            }
                            if "debug" in inst:
                                nop["debug"] = inst["debug"]
                            out.append(nop)
                        si["on_wait"] = waits[-1:]
                out.append(inst)
            blk["instructions"] = out
    return _json.dumps(d) if changed else bir_json


def _patched_compile_bir_kernel(bir_json, tmpdir, neff_name="file.neff"):
    return _orig_compile_bir_kernel(_fix_bir(bir_json), tmpdir, neff_name=neff_name)


def _install_waitfix():
    bass_utils.compile_bir_kernel = _patched_compile_bir_kernel
    bass2jax.compile_bir_kernel = _patched_compile_bir_kernel


_install_waitfix()

# ---------------------------------------------------------------- program


def build_program():
    """One SPMD Bass program; per-core differences come in via inputs."""
    nc = bass.Bass()

    hiddenT = nc.dram_tensor("hiddenT", [D, S], BF, kind="ExternalInput")
    w_qkv = nc.dram_tensor("w_qkv", [D, 768], BF, kind="ExternalInput")
    bqk = nc.dram_tensor("bqk", [128, 4], F32, kind="ExternalInput")
    bv_rep = nc.dram_tensor("bv_rep", [128, 256], F32, kind="ExternalInput")
    projw = nc.dram_tensor("projw", [128, 2, D], BF, kind="ExternalInput")
    diag_mask = nc.dram_tensor("diag_mask", [128, 128], BF, kind="ExternalInput")
    ident = nc.dram_tensor("ident", [128, 128], BF, kind="ExternalInput")
    logmult = nc.dram_tensor("logmult", [128, HPC, NT], F32, kind="ExternalInput")
    out = nc.dram_tensor("out", [S, D], BF, kind="ExternalOutput")

    with tile.TileContext(nc) as tc:
        with tc.tile_pool(name="persist", bufs=1) as persist, \
             tc.tile_pool(name="atp", bufs=2) as atp, \
             tc.tile_pool(name="stgp", bufs=2) as stgp, \
             tc.tile_pool(name="recp", bufs=4) as recp, \
             tc.tile_pool(name="outp", bufs=4) as outp:

            # ---- persistent SBUF ----
            # hT and w_sb die after window 1 (last v-round / qk-r1 reads);
            # head 2's attention buffer manually reuses their bytes via
            # overlapping views of one scratch tile (Tile's AP-range WAR
            # tracking orders the reuse).
            scratch = persist.tile([128, 23552], BF)      # 5.75 MB
            hT = scratch[:, 0:KO * S].rearrange("p (ko s) -> p ko s", s=S)
            w_sb = scratch[:, KO * S:KO * S + KO * 768].rearrange(
                "p (ko n) -> p ko n", n=768)
            at2_view = scratch[:, 0:ATW]
            at3_sb = persist.tile([128, ATW], BF)         # head 3, no WAR
            qk_sb = persist.tile([128, 4, S], BF)         # 2 MB
            v_sb = persist.tile([128, NT, HPC, 65], BF)   # ~1 MB
            ao2 = persist.tile([128, 2, S], BF)           # attn_outT, 1 MB
            bqk_sb = persist.tile([128, 4], F32)
            bv_sb = persist.tile([128, 256], F32)
            pw_sb = persist.tile([128, 2, D], BF)         # 0.5 MB
            dm_sb = persist.tile([128, 128], BF)
            id_sb = persist.tile([128, 128], BF)
            lm_sb = persist.tile([128, HPC, NT], F32)

            nc.sync.dma_start(bqk_sb, bqk[:, :])
            nc.vector.memset(v_sb[:, :, :, 64:65], 1.0)

            # input loads: hT alternates the SP and ACT HWDGE queues (ACT
            # is idle until the first exp), w + small tensors ride the
            # GPSIMD SWDGE queue.
            hT_src = hiddenT.rearrange("(ko p) s -> p ko s", p=128)
            w_src = w_qkv.rearrange("(ko p) n -> p ko n", p=128)
            # three-way load split tuned so each qk chunk lands just
            # before the PE consumes it: SP and ACT stream hT (first
            # chunks in small pieces), the SWDGE queue delivers the
            # pass-1-critical first 128 w columns ahead of the rest.
            nc.gpsimd.dma_start(w_sb[:, 0, 0:128], w_src[:, 0, 0:128])
            for pc in range(4):
                nc.sync.dma_start(hT[:, 0, 512 * pc:512 * pc + 512],
                                  hT_src[:, 0, 512 * pc:512 * pc + 512])
            for pc in range(2):
                nc.scalar.dma_start(hT[:, 1, 1024 * pc:1024 * pc + 1024],
                                    hT_src[:, 1, 1024 * pc:1024 * pc + 1024])
            nc.gpsimd.dma_start(w_sb[:, 1:KO, 0:128], w_src[:, 1:KO, 0:128])
            nc.scalar.dma_start(hT[:, 2, :], hT_src[:, 2, :])
            nc.sync.dma_start(hT[:, 3, :], hT_src[:, 3, :])
            nc.gpsimd.dma_start(hT[:, 4, :], hT_src[:, 4, :])
            nc.scalar.dma_start(hT[:, 5, :], hT_src[:, 5, :])
            nc.sync.dma_start(hT[:, 6, :], hT_src[:, 6, :])
            nc.scalar.dma_start(hT[:, 7, :], hT_src[:, 7, :])
            nc.gpsimd.dma_start(w_sb[:, :, 128:768], w_src[:, :, 128:768])
            nc.sync.dma_start(bv_sb, bv_rep[:, :])
            nc.gpsimd.dma_start(dm_sb, diag_mask[:, :])
            nc.gpsimd.dma_start(id_sb, ident[:, :])
            nc.gpsimd.dma_start(lm_sb, logmult[:, :, :])
            nc.gpsimd.dma_start(pw_sb, projw[:, :, :])

            # ========== qk projection round 0 (heads 0/1: nt 0 and 2) =====
            # ko (contraction) outer with 4 resident PSUM groups per pass;
            # two passes so the first pass's banks drain (DVE) while the
            # second computes, letting the attention pools start without
            # waiting for the full round to drain.
            with tc.tile_pool(name="p1ps", bufs=7, space="PSUM") as p1ps:
                # prologue computes ONLY q of heads 0/1 (nt 0): scores for
                # k-tile t need just one 512-column k chunk, so the k
                # projection is deferred into window 0 as PE filler and
                # the first exp fires ~7us earlier.
                ps_kb = p1ps.tile([128, 512], F32, tag="g", name="kb0")
                ps_vb = [p1ps.tile([128, 512], F32, tag="g", name=f"vb{st}")
                         for st in range(2)]
                ps4 = [p1ps.tile([128, 512], F32, tag="g", name=f"q0{i}")
                       for i in range(4)]
                for ko in range(KO):
                    for i in range(4):
                        nc.tensor.matmul(
                            ps4[i],
                            w_sb[:, ko, 0:128],
                            hT[:, ko, 512 * i:512 * i + 512],
                            start=(ko == 0), stop=(ko == KO - 1),
                        )
                for i in range(4):
                    if i % 2 == 0:
                        nc.vector.tensor_scalar_add(
                            qk_sb[:, 0, 512 * i:512 * i + 512], ps4[i],
                            bqk_sb[:, 0:1],
                        )
                    else:
                        nc.scalar.activation(
                            qk_sb[:, 0, 512 * i:512 * i + 512], ps4[i],
                            mybir.ActivationFunctionType.Identity,
                            bias=bqk_sb[:, 0:1], scale=1.0,
                        )
                # first k chunk + first two v tiles bridge the pool
                # boundary out of drained pass-1 slots
                ps = ps_kb
                for ko in range(KO):
                    nc.tensor.matmul(
                        ps,
                        w_sb[:, ko, 256:384],
                        hT[:, ko, 0:512],
                        start=(ko == 0), stop=(ko == KO - 1),
                    )
                nc.scalar.activation(
                    qk_sb[:, 2, 0:512], ps,
                    mybir.ActivationFunctionType.Identity,
                    bias=bqk_sb[:, 2:3], scale=1.0,
                )
                for st in range(2):
                    ps = ps_vb[st]
                    for ko in range(KO):
                        nc.tensor.matmul(
                            ps[:, 0:256],
                            hT[:, ko, 128 * st:128 * st + 128],
                            w_sb[:, ko, 512:768],
                            start=(ko == 0), stop=(ko == KO - 1),
                        )
                    nc.vector.tensor_add(
                        out=v_sb[:, st, :, 0:64],
                        in0=ps[:, 0:256].rearrange("p (h d) -> p h d", d=64),
                        in1=bv_sb.rearrange("p (h d) -> p h d", d=64),
                    )

            # ================= attention + everything else =================
            # pool order fixes which freed qk-round banks each tag aliases:
            # ppr first so the w0 v-rounds can start as soon as the first
            # qk pass drains.  av/tp are single-bank tiles whose 65/128-col
            # sub-slices rotate manually: depth-4 pipelining in one bank
            # each (PSUM allocation is bank-granular, so separate tiles
            # would blow the 8-bank budget).
            with tc.tile_pool(name="ppr", bufs=2, space="PSUM") as ppr, \
                 tc.tile_pool(name="psc", bufs=2, space="PSUM") as psc, \
                 tc.tile_pool(name="pav", bufs=1, space="PSUM") as pav, \
                 tc.tile_pool(name="ptp", bufs=1, space="PSUM") as ptp:

                av_big = pav.tile([128, 260], F32)
                tp_big = ptp.tile([128, 512], BF)
                stg_by = {}

                def emit_scores(lh, t):
                    """score pieces + exp into at bufs, then diag mask."""
                    bp = 64 * (lh % 2)
                    qn, kn = lh // 2, 2 + lh // 2
                    at_sb = at_by[lh]
                    gs = 128 * t
                    while gs < S:
                        w = min(1024, S - gs)
                        sc = psc.tile([128, 1024], F32, tag="sc")
                        o = 0
                        while o < w:
                            n = min(512, w - o)
                            nc.tensor.matmul(
                                sc[:, o:o + n],
                                qk_sb[bp:bp + 64, kn, 128 * t:128 * t + 128],
                                qk_sb[bp:bp + 64, qn, gs + o:gs + o + n],
                                start=True, stop=True,
                            )
                            o += n
                        nc.scalar.activation(
                            at_sb[:, OFF[t] + gs - 128 * t:
                                  OFF[t] + gs - 128 * t + w],
                            sc[:, :w],
                            mybir.ActivationFunctionType.Exp,
                            bias=lm_sb[:, lh, t:t + 1], scale=0.125,
                        )
                        if gs == 128 * t:
                            # causal 0/1 mask on the diagonal block
                            # (GPSIMD, all-SBUF): emitted right after the
                            # first piece so the AV diagonal never waits
                            nc.gpsimd.tensor_mul(
                                out=at_sb[:, OFF[t]:OFF[t] + 128],
                                in0=at_sb[:, OFF[t]:OFF[t] + 128],
                                in1=dm_sb,
                            )
                        gs += w

                av_rr = [0]
                tp_rr = [0]

                def emit_av(lh, t):
                    """flipped AV for q-tile t + per-partition normalize."""
                    at_sb = at_by[lh]
                    c = av_rr[0] % 4
                    av_rr[0] += 1
                    av = av_big[:, 65 * c:65 * c + 65]
                    for u in range(t + 1):
                        nc.tensor.matmul(
                            av,
                            at_sb[:, OFF[u] + 128 * (t - u):
                                  OFF[u] + 128 * (t - u) + 128],
                            v_sb[:, u, lh, :],
                            start=(u == 0), stop=(u == t),
                        )
                    rec = recp.tile([128, 1], F32, tag="rec")
                    nc.vector.reciprocal(rec, av[:, 64:65])
                    nc.vector.tensor_scalar_mul(
                        stg_by[lh // 2][:, t, lh % 2, :], av[:, 0:64], rec,
                    )

                def emit_transpose(j, st, on_act=False):
                    """O' [q,(e,d)] block -> ao2 [(e,d), q] via PE."""
                    c = tp_rr[0] % 4
                    tp_rr[0] += 1
                    tp = tp_big[:, 128 * c:128 * c + 128]
                    nc.tensor.matmul(
                        tp, stg_by[j][:, st, :, :], id_sb,
                        start=True, stop=True, is_transpose=True,
                    )
                    if on_act:
                        nc.scalar.copy(ao2[:, j, 128 * st:128 * st + 128], tp)
                    else:
                        nc.vector.tensor_copy(
                            ao2[:, j, 128 * st:128 * st + 128], tp,
                        )

                def emit_cproj(st, ec, on_act=False):
                    pr = ppr.tile([128, 512], F32, tag="pr")
                    for jj in range(2):
                        nc.tensor.matmul(
                            pr,
                            ao2[:, jj, 128 * st:128 * st + 128],
                            pw_sb[:, jj, 512 * ec:512 * ec + 512],
                            start=(jj == 0), stop=(jj == 1),
                        )
                    o_sb = outp.tile([128, 512], BF, tag="osb")
                    if on_act:
                        nc.scalar.copy(o_sb, pr)
                    else:
                        nc.vector.tensor_copy(o_sb, pr)
                    oq = nc.scalar if st >= NT - 2 else nc.sync
                    oq.dma_start(
                        out[128 * st:128 * st + 128,
                            512 * ec:512 * ec + 512],
                        o_sb,
                    )

                def emit_vround(st):
                    """v natural: out[s-tile, (h,d)] = hidden @ wv."""
                    ps = ppr.tile([128, 512], F32, tag="pr", name=f"v{st}")
                    for ko in range(KO):
                        nc.tensor.matmul(
                            ps[:, 0:256],
                            hT[:, ko, 128 * st:128 * st + 128],
                            w_sb[:, ko, 512:768],
                            start=(ko == 0), stop=(ko == KO - 1),
                        )
                    nc.vector.tensor_add(
                        out=v_sb[:, st, :, 0:64],
                        in0=ps[:, 0:256].rearrange("p (h d) -> p h d", d=64),
                        in1=bv_sb.rearrange("p (h d) -> p h d", d=64),
                    )

                def emit_proj(nt, sc4):
                    """one deferred qk projection chunk (ko-inner)."""
                    ps = ppr.tile([128, 512], F32, tag="pr",
                                  name=f"qp{nt}{sc4}")
                    for ko in range(KO):
                        nc.tensor.matmul(
                            ps,
                            w_sb[:, ko, 128 * nt:128 * nt + 128],
                            hT[:, ko, 512 * sc4:512 * sc4 + 512],
                            start=(ko == 0), stop=(ko == KO - 1),
                        )
                    nc.vector.tensor_scalar_add(
                        qk_sb[:, nt, 512 * sc4:512 * sc4 + 512], ps,
                        bqk_sb[:, nt:nt + 1],
                    )

                at_by = {}

                def new_at(lh):
                    if lh == 2:
                        at_by[2] = at2_view
                    elif lh == 3:
                        at_by[3] = at3_sb
                    else:
                        at_by[lh] = atp.tile([128, ATW], BF, tag="at",
                                             name=f"at{lh}")
                    if lh // 2 not in stg_by:
                        stg_by[lh // 2] = stgp.tile([128, NT, 2, 64], BF,
                                                    tag="stg",
                                                    name=f"stg{lh // 2}")

                # ============ global greedy PE/ACT budgeter ============
                # One global stream of score rounds (heads 0+1 interleaved,
                # then 3+2 staggered).  AV / transpose / c_proj chase
                # dependency pointers.  A tiny time model runs alongside:
                # pe_t = PE busy time emitted so far (the emission order IS
                # the PE execution order), act_end = when ScalarE finishes
                # everything queued so far.  Fillers (v rounds, deferred
                # projections) drain whenever ScalarE's backlog covers
                # their PE time, so ScalarE never starves; hard deadlines
                # protect correctness.  Late c_proj/transpose drain copies
                # ride ScalarE, which is idle once the exps run out.
                clk = {"pe": 10200.0, "act": 0.0}

                def c_scores(lh, t):
                    emit_scores(lh, t)
                    w = S - 128 * t
                    clk["pe"] += w * 0.4167
                    clk["act"] = max(clk["act"], clk["pe"]) \
                        + w * 0.833 + 200.0 * ((w + 1023) // 1024)

                def c_av(lh, t):
                    emit_av(lh, t)
                    clk["pe"] += 27.1 * (t + 1)

                def c_T(j, st):
                    late = j == 1 and st >= 12
                    emit_transpose(j, st, on_act=late)
                    clk["pe"] += 53.0
                    if late:
                        clk["act"] = max(clk["act"], clk["pe"]) + 350.0

                def c_cp(st):
                    late = st >= 10
                    emit_cproj(st, 0, on_act=False)
                    emit_cproj(st, 1, on_act=late)
                    clk["pe"] += 854.0
                    if late:
                        clk["act"] = max(clk["act"], clk["pe"]) + 610.0

                # ---- job list ----
                jobs = []
                for t in range(NT):
                    jobs.append((0, t))
                    jobs.append((1, t))
                for t in range(NT):
                    jobs.append((3, t))
                    if t >= 4:
                        jobs.append((2, t - 4))
                for u in range(NT - 4, NT):
                    jobs.append((2, u))
                ji = {}
                for i, j in enumerate(jobs):
                    ji[j] = i
                J_B = ji[(3, 0)]
                J_H2 = ji[(2, 0)]

                # ---- filler queue: (deadline job idx, emit fn).
                # Deadlines are per-consumer: k chunk c before scores
                # (0/1, 4c); v(u) before the first AV that reads it and in
                # any case before head 2's first exp overwrites hT; the
                # nt1 chunks + nt3 chunk 0 before head 3 starts, the other
                # nt3 chunks before head 3's tile 4c. ----
                vd = [1]    # v tiles 0..vd[0] exist (v0/v1 from prologue)

                def emit_v_count(u):
                    emit_vround(u)
                    vd[0] = u

                fillers = []
                for c in range(1, 4):
                    fillers.append((8 * c,
                                    (lambda c=c: emit_proj(2, c), 1707.0)))
                for g in range(4):
                    fillers.append((J_B - 1,
                                    (lambda g=g: emit_proj(1, g), 1707.0)))
                fillers.append((J_B - 1, (lambda: emit_proj(3, 0), 1707.0)))
                for c in range(1, 4):
                    fillers.append((ji[(3, 4 * c)] - 1,
                                    (lambda c=c: emit_proj(3, c), 1707.0)))
                # v rounds gate only head 2 (hT overlay WAR) and the
                # head-0/1 AV chase, which is capped by vd
                for u in range(2, NT):
                    fillers.append((min(J_B + u, J_H2 - 1),
                                    (lambda u=u: emit_v_count(u), 853.0)))
                fillers.sort(key=lambda x: x[0])
                fillers = fillers[::-1]   # pop from the end

                scored = {}
                avd = {0: -1, 1: -1, 2: -1, 3: -1}
                t0d, t1d, cpd = [-1], [-1], [-1]

                def av_tgt(s):
                    return max(min(s - 1, 2), s - 3 if s < 11 else s - 2)

                def chase(budget_hungry):
                    # mandatory dependency-paced work, at most a step each
                    for lh in (0, 1, 3, 2):
                        if lh in scored:
                            tgt = min(av_tgt(scored[lh]), NT - 1)
                            if lh < 2:
                                tgt = min(tgt, vd[0])
                            while avd[lh] < tgt:
                                avd[lh] += 1
                                c_av(lh, avd[lh])
                    if t0d[0] < min(avd[0], avd[1]) - 1:
                        t0d[0] += 1
                        c_T(0, t0d[0])
                    if t1d[0] < min(avd[3], avd[2]) - 1:
                        t1d[0] += 1
                        c_T(1, t1d[0])
                    ncp = 2 if budget_hungry else 1
                    for _ in range(ncp):
                        if cpd[0] < min(t0d[0], t1d[0]) - 1:
                            cpd[0] += 1
                            c_cp(cpd[0])

                new_at(0)
                new_at(1)
                for i, (lh, t) in enumerate(jobs):
                    if (lh, t) == (2, 0):
                        new_at(2)
                    if i == J_B:
                        new_at(3)
                    c_scores(lh, t)
                    scored[lh] = max(scored.get(lh, -1), t)
                    chase(clk["act"] > clk["pe"] + 2500.0)
                    # deadline-forced fillers, then backlog-driven drain:
                    # a filler is free when ScalarE still has that much
                    # queued work to chew through
                    while fillers and fillers[-1][0] <= i:
                        _, (fn, pe) = fillers.pop()
                        fn()
                        clk["pe"] += pe
                    if fillers and i >= J_B:
                        # in window B ScalarE runs a real backlog the clock
                        # model can't see (it idled through the boundary
                        # dump): drain one filler per job unconditionally
                        _, (fn, pe) = fillers.pop()
                        fn()
                        clk["pe"] += pe
                    while fillers and \
                            clk["act"] - clk["pe"] > fillers[-1][1][1] + 300:
                        _, (fn, pe) = fillers.pop()
                        fn()
                        clk["pe"] += pe
                # ---- tail: drain all remaining pointer work ----
                for _ in range(3 * NT):
                    before = (tuple(avd.values()), t0d[0], t1d[0], cpd[0])
                    for lh in (0, 1, 3, 2):
                        if lh in scored and avd[lh] < NT - 1:
                            avd[lh] += 1
                            c_av(lh, avd[lh])
                    if t0d[0] < min(avd[0], avd[1]) - 1 or \
                       (min(avd[0], avd[1]) == NT - 1 and t0d[0] < NT - 1):
                        t0d[0] += 1
                        c_T(0, t0d[0])
                    if t1d[0] < min(avd[3], avd[2]) - 1 or \
                       (min(avd[3], avd[2]) == NT - 1 and t1d[0] < NT - 1):
                        t1d[0] += 1
                        c_T(1, t1d[0])
                    lim = min(t0d[0], t1d[0])
                    lim = lim if lim < NT - 1 else NT - 1
                    for _ in range(2):
                        if cpd[0] < lim - (0 if lim == NT - 1 else 1):
                            cpd[0] += 1
                            c_cp(cpd[0])
                    after = (tuple(avd.values()), t0d[0], t1d[0], cpd[0])
                    if before == after and cpd[0] >= NT - 1:
                        break
                while cpd[0] < NT - 1:
                    cpd[0] += 1
                    c_cp(cpd[0])
    return nc


_NC = None


def _get_nc():
    global _NC
    if _NC is None:
        _NC = build_program()
    return _NC


# ---------------------------------------------------------------- host prep

def make_in_maps(hidden_states, c_attn_w, c_attn_b, c_proj_w):
    first_end = S // 3
    second_end = 2 * S // 3
    pos = np.arange(S)
    regions = [pos < first_end,
               (pos >= first_end) & (pos < second_end),
               pos >= second_end]
    mult = np.ones((H, S), dtype=np.float64)
    for h, r in HEAD_REGION.items():
        mult[h] = 1.0 + (FOCUS - 1.0) * regions[r].astype(np.float64)
    logm = np.log(mult).astype(np.float32)  # [H, S]

    p = np.arange(128)[:, None]
    j = np.arange(128)[None, :]
    diag = (j >= p).astype(np.float32)  # 0/1 keep-mask, applied post-exp
    iden = (j == p).astype(np.float32)

    bf = ml_dtypes.bfloat16
    in_maps = []
    for c in range(NCORES):
        b, g = divmod(c, GROUPS)
        h0 = HPC * g
        cs = slice(256 * g, 256 * g + 256)
        w_qkv = np.concatenate(
            [c_attn_w[:, cs], c_attn_w[:, 1024:2048][:, cs],
             c_attn_w[:, 2048:3072][:, cs]], axis=1,
        ).astype(bf)
        bqk_h = np.concatenate(
            [c_attn_b[cs], c_attn_b[1024:2048][cs]]
        ).reshape(4, 128).T.copy().astype(np.float32)
        bv = np.broadcast_to(
            c_attn_b[2048:3072][cs], (128, 256)
        ).astype(np.float32).copy()
        # pw2[p, j, e]: head pair j=(2j, 2j+1); p<64 -> head 2j row p,
        # p>=64 -> head 2j+1 row p-64  (matches ao2 partition interleave)
        pw = c_proj_w[64 * h0:64 * h0 + 256, :].reshape(2, 128, D)
        pw = np.ascontiguousarray(pw.transpose(1, 0, 2)).astype(bf)
        lm = logm[h0:h0 + HPC].reshape(HPC, S // 128, 128)
        lm = np.ascontiguousarray(lm.transpose(2, 0, 1)).astype(np.float32)
        in_maps.append({
            "hiddenT": np.ascontiguousarray(hidden_states[b].T).astype(bf),
            "w_qkv": w_qkv,
            "bqk": bqk_h,
            "bv_rep": bv,
            "projw": pw,
            "diag_mask": diag.astype(bf),
            "ident": iden.astype(bf),
            "logmult": lm,
        })
    return in_maps


def run_cores(in_maps, trace=False, **kw):
    from concourse.bass_utils import run_bass_kernel_spmd
    nc = _get_nc()
    return run_bass_kernel_spmd(nc, in_maps, core_ids=list(range(NCORES)),
                                trace=trace, **kw)


def kernel(hidden_states, c_attn_w, c_attn_b, c_proj_w, c_proj_b):
    hidden_states = np.asarray(hidden_states, dtype=np.float32)
    c_attn_w = np.asarray(c_attn_w, dtype=np.float32)
    c_attn_b = np.asarray(c_attn_b, dtype=np.float32)
    c_proj_w = np.asarray(c_proj_w, dtype=np.float32)
    c_proj_b = np.asarray(c_proj_b, dtype=np.float32)

    in_maps = make_in_maps(hidden_states, c_attn_w, c_attn_b, c_proj_w)
    res = run_cores(in_maps)
    out = np.zeros((B, S, D), dtype=np.float32)
    for c in range(NCORES):
        out[c // GROUPS] += np.asarray(res.results[c]["out"],
                                       dtype=np.float32)
    out += c_proj_b[None, None, :]
    return out
